# revision 1
# baseline (speedup 1.0000x reference)
"""Trainium2 Bass kernel for nn_Attention (softmax(tanh(key @ (W @ query) + bias))).

Shapes (full): query [64, 512], key [64, 2048, 512], W [512, 512], bias [1].
Output: softmax over T of tanh(einsum('btk,bk->bt', key, W@query^T per batch) + bias).

Sharding: data-parallel over batch B=64 across 8 cores (8 batches/core);
W and bias replicated.

Per-core design (DMA-roofline bound: 32 MB of key per core @ ~358 GB/s):
  - mids[b] = W @ query[b] computed on TensorE in true fp32 (small).
  - mids broadcast across 128 partitions via a selector matmul (TensorE).
  - z[b, t] = sum_k key[b,t,k] * mids[b,k] runs on VectorE (and optionally
    GpSimd) as ONE fused scalar_tensor_tensor per [128t, 512k] tile in the
    natural DMA layout (t on partitions, k on the free axis), accumulating
    into accum_out in fp32. No transposes of the big key tensor at all.
  - tanh (+bias) and exp (+free-axis sum) on ScalarE; partition sums and the
    final [128,16] -> [16,128] output transpose on TensorE; reciprocal and
    normalization on VectorE. Softmax needs no max-subtraction: tanh output
    is in (-1, 1) so exp is in (e^-1, e).
"""

from contextlib import ExitStack

import numpy as np

import concourse.bacc as bacc
import concourse.mybir as mybir
import concourse.tile as tile
from concourse import masks
from concourse.bass_utils import run_bass_kernel_spmd

F32 = mybir.dt.float32
MULT = mybir.AluOpType.mult
AF = mybir.ActivationFunctionType

N_CORES = 8
B, T, Q, K = 64, 2048, 512, 512
B_LOC = B // N_CORES          # 8 batches per core
N_TBLK = T // 128             # 16 [128, K] tiles per batch
N_CHUNK = 4                   # DMA chunks per batch
KEY_BUFS = 8                  # key tile pool depth
# t-blocks per chunk handled by GpSimd instead of VectorE. Must stay 0: walrus
# rejects TensorScalarPtr on the Pool engine (not in the TRN2 ISA for GpSimd).
N_GPSIMD_PER_CHUNK = 0
# --- cost-model probe knobs (must be default for correctness) ---
STT_K = 512        # contraction length seen by the STT (probe only)
STT_FAKE = False   # STT reads mids instead of key (decouples DMA from DVE)
NO_STT = False     # skip the STT entirely (memset z once)
REPS = None        # if set, wrap the main loop in a hardware For_i (timing only)


def emit(tc, ctx):
    nc = tc.nc
    query = nc.dram_tensor("query", [B_LOC, Q], F32, kind="ExternalInput").ap()
    key = nc.dram_tensor("key", [B_LOC, T, K], F32, kind="ExternalInput").ap()
    W = nc.dram_tensor("W", [K, Q], F32, kind="ExternalInput").ap()
    bias = nc.dram_tensor("bias", [1, 1], F32, kind="ExternalInput").ap()
    out = nc.dram_tensor("out", [B_LOC, T], F32, kind="ExternalOutput").ap()

    TBLK_PER_CHUNK = N_TBLK // N_CHUNK
    KC = K // 128  # 4 chunks of the k axis
    QC = Q // 128  # 4 chunks of the q axis

    const = ctx.enter_context(tc.tile_pool(name="const", bufs=1))
    key_pool = ctx.enter_context(tc.tile_pool(name="keyp", bufs=KEY_BUFS))
    z_pool = ctx.enter_context(tc.tile_pool(name="zp", bufs=2))
    ep_pool = ctx.enter_context(tc.tile_pool(name="epp", bufs=2))
    ps_setup = ctx.enter_context(tc.tile_pool(name="pss", bufs=2, space="PSUM"))
    ps_main = ctx.enter_context(tc.tile_pool(name="psm", bufs=3, space="PSUM"))

    # ---- constants ----
    identity = const.tile([128, 128], F32, tag="identity")
    masks.make_identity(nc, identity[:])
    ones_col = const.tile([128, 1], F32, tag="ones_col")
    nc.vector.memset(ones_col[:], 1.0)
    ones_row = const.tile([1, 128], F32, tag="ones_row")
    nc.vector.memset(ones_row[:], 1.0)
    sel = const.tile([B_LOC, B_LOC, 128], F32, tag="sel")
    nc.gpsimd.memset(sel[:], 0.0)
    # sel[c, b, p] = 1.0 where c == b (selector columns for the mids broadcast)
    nc.gpsimd.affine_select(
        out=sel[:],
        in_=sel[:],
        compare_op=mybir.AluOpType.not_equal,
        fill=1.0,
        base=0,
        pattern=[[-1, B_LOC], [0, 128]],
        channel_multiplier=1,
    )

    # ---- small inputs (ACT HWDGE queue; key uses the sync queue) ----
    W_sb = const.tile([128, KC, Q], F32, tag="W_sb")
    nc.scalar.dma_start(out=W_sb[:], in_=W.rearrange("(kc p) q -> p kc q", p=128))
    q_sb = const.tile([B_LOC, Q], F32, tag="q_sb")
    nc.scalar.dma_start(out=q_sb[:], in_=query)
    bias_sb = const.tile([1, 1], F32, tag="bias_sb")
    nc.scalar.dma_start(out=bias_sb[:], in_=bias)

    # ---- W^T via TensorE transposes: WT_sb[p, qc, k] = W[k, qc*128+p] ----
    WT_sb = const.tile([128, QC, K], F32, tag="WT_sb")
    for qc in range(QC):
        wt_ps = ps_setup.tile([128, K], F32, tag="s")
        for kc in range(KC):
            nc.tensor.transpose(
                wt_ps[:, kc * 128 : (kc + 1) * 128],
                W_sb[:, kc, qc * 128 : (qc + 1) * 128],
                identity[:],
            )
        nc.scalar.copy(WT_sb[:, qc, :], wt_ps[:])

    # ---- query^T: qT_sb[p, qc, b] = query[b, qc*128+p] ----
    qT_sb = const.tile([128, QC, B_LOC], F32, tag="qT_sb")
    for qc in range(QC):
        qt_ps = ps_setup.tile([128, B_LOC], F32, tag="s")
        nc.tensor.transpose(
            qt_ps[:],
            q_sb[:, qc * 128 : (qc + 1) * 128],
            identity[:B_LOC, :B_LOC],
        )
        nc.vector.tensor_copy(qT_sb[:, qc, :], qt_ps[:])

    # ---- mids[b, k] = sum_q W[k, q] query[b, q]  (true fp32 matmul) ----
    mids_ps = ps_setup.tile([B_LOC, K], F32, tag="s")
    for qc in range(QC):
        nc.tensor.matmul(
            mids_ps[:],
            qT_sb[:, qc, :],
            WT_sb[:, qc, :],
            start=(qc == 0),
            stop=(qc == QC - 1),
        )
    mids_sb = const.tile([B_LOC, K], F32, tag="mids_sb")
    nc.scalar.copy(mids_sb[:], mids_ps[:])

    # ---- bias broadcast to [128, 1] ----
    bb_ps = ps_setup.tile([128, 1], F32, tag="s")
    nc.tensor.matmul(bb_ps[:], ones_row[:], bias_sb[:], start=True, stop=True)
    bias_bc = const.tile([128, 1], F32, tag="bias_bc")
    nc.vector.tensor_copy(bias_bc[:], bb_ps[:])

    # ---- mids broadcast: mids_bc[p, b, k] = mids[b, k] for all p ----
    mids_bc = const.tile([128, B_LOC, K], F32, tag="mids_bc")
    for b in range(B_LOC):
        bc_ps = ps_setup.tile([128, K], F32, tag="s")
        nc.tensor.matmul(bc_ps[:], sel[:, b, :], mids_sb[:], start=True, stop=True)
        nc.scalar.copy(mids_bc[:, b, :], bc_ps[:])

    # ---- main loop ----
    scratch_v = const.tile([128, K], F32, tag="scratch_v")
    scratch_g = const.tile([128, K], F32, tag="scratch_g")
    out_sb = const.tile([N_TBLK, B_LOC, 128], F32, tag="out_sb")

    key_r = key.rearrange("b (c n p) k -> b c p n k", n=TBLK_PER_CHUNK, p=128)

    def main_body():
        main_loop(tc, nc, key_pool, z_pool, ep_pool, ps_main, key_r,
                  mids_bc, bias_bc, scratch_v, scratch_g, out_sb,
                  identity, ones_col, ones_row, TBLK_PER_CHUNK)

    if REPS is None:
        main_body()
    else:
        with tc.For_i(0, REPS, 1):
            main_body()

    nc.scalar.dma_start(
        out=out.rearrange("b (n p) -> n b p", p=128), in_=out_sb[:]
    )


def main_loop(tc, nc, key_pool, z_pool, ep_pool, ps_main, key_r,
              mids_bc, bias_bc, scratch_v, scratch_g, out_sb,
              identity, ones_col, ones_row, TBLK_PER_CHUNK):
    for b in range(B_LOC):
        z_t = z_pool.tile([128, N_TBLK], F32, tag="z")
        if NO_STT:
            nc.vector.memset(z_t[:], 0.5)
        for c in range(N_CHUNK):
            key_t = key_pool.tile([128, TBLK_PER_CHUNK, K], F32, tag="key")
            nc.sync.dma_start(out=key_t[:], in_=key_r[b, c])
            for n in range(TBLK_PER_CHUNK):
                j = c * TBLK_PER_CHUNK + n
                if NO_STT:
                    continue
                if n < TBLK_PER_CHUNK - N_GPSIMD_PER_CHUNK:
                    eng, scratch = nc.vector, scratch_v
                else:
                    eng, scratch = nc.gpsimd, scratch_g
                in0 = mids_bc[:, b, :STT_K] if STT_FAKE else key_t[:, n, :STT_K]
                eng.scalar_tensor_tensor(
                    out=scratch[:, :STT_K],
                    in0=in0,
                    scalar=1.0,
                    in1=mids_bc[:, b, :STT_K],
                    op0=MULT,
                    op1=MULT,
                    accum_out=z_t[:, j : j + 1],
                )

        th = ep_pool.tile([128, N_TBLK], F32, tag="th")
        nc.scalar.activation(th[:], z_t[:], AF.Tanh, bias=bias_bc[:], scale=1.0)
        ex = ep_pool.tile([128, N_TBLK], F32, tag="ex")
        exsum = ep_pool.tile([128, 1], F32, tag="exsum")
        nc.scalar.activation(ex[:], th[:], AF.Exp, accum_out=exsum[:])

        sum_ps = ps_main.tile([1, 1], F32, tag="m")
        nc.tensor.matmul(sum_ps[:], exsum[:], ones_col[:], start=True, stop=True)
        rec_sb = ep_pool.tile([1, 1], F32, tag="rec")
        nc.vector.reciprocal(rec_sb[:], sum_ps[:])
        rb_ps = ps_main.tile([128, 1], F32, tag="m")
        nc.tensor.matmul(rb_ps[:], ones_row[:], rec_sb[:], start=True, stop=True)
        rb_sb = ep_pool.tile([128, 1], F32, tag="rb")
        nc.vector.tensor_copy(rb_sb[:], rb_ps[:])

        norm = ep_pool.tile([128, N_TBLK], F32, tag="norm")
        nc.vector.tensor_scalar_mul(norm[:], ex[:], rb_sb[:])

        outT_ps = ps_main.tile([N_TBLK, 128], F32, tag="m")
        nc.tensor.transpose(outT_ps[:], norm[:], identity[:])
        nc.scalar.copy(out_sb[:, b, :], outT_ps[:])


_NC_CACHE = None


def build():
    global _NC_CACHE
    if _NC_CACHE is None:
        nc = bacc.Bacc(trn_type="TRN2", enable_partition_id=False)
        with tile.TileContext(nc) as tc:
            with ExitStack() as ctx:
                emit(tc, ctx)
        nc.compile()
        _NC_CACHE = nc
    return _NC_CACHE


def kernel(**inputs) -> np.ndarray:
    query = np.ascontiguousarray(np.asarray(inputs["query"], dtype=np.float32))
    key = np.ascontiguousarray(np.asarray(inputs["key"], dtype=np.float32))
    W = np.ascontiguousarray(np.asarray(inputs["W"], dtype=np.float32))
    bias = np.asarray(inputs["bias"], dtype=np.float32).reshape(1, 1)

    nc = build()
    in_maps = []
    for c in range(N_CORES):
        lo, hi = c * B_LOC, (c + 1) * B_LOC
        in_maps.append(
            {
                "query": np.ascontiguousarray(query[lo:hi]),
                "key": np.ascontiguousarray(key[lo:hi]),
                "W": W,
                "bias": bias,
            }
        )
    res = run_bass_kernel_spmd(nc, in_maps, core_ids=list(range(N_CORES)))
    return np.concatenate([res.results[c]["out"] for c in range(N_CORES)], axis=0)



# revision 21
# speedup vs baseline: 1.6795x; 1.6795x over previous
"""Trainium2 Bass kernel for nn_Attention (softmax(tanh(key @ (W @ query) + bias))).

Shapes (full): query [64, 512], key [64, 2048, 512], W [512, 512], bias [1].
Output: softmax over T of tanh(einsum('btk,bk->bt', key, W@query^T per batch) + bias).

Sharding: data-parallel over batch B=64 across 8 cores (8 batches/core);
W and bias replicated.

Per-core design (DMA-roofline bound: 32 MB of key per core @ ~358 GB/s):
  - mids[b] = W @ query[b] computed on TensorE in true fp32 (small).
  - mids broadcast across 128 partitions via a selector matmul (TensorE).
  - key is loaded p-major: within each chunk, partition p holds CHUNK_TBLK
    consecutive t rows (t = c*128*CHUNK_TBLK + p*CHUNK_TBLK + n), giving ONE
    contiguous 16 KB DRAM descriptor per partition per chunk. Chunks alternate
    between the two HWDGE queues (sync/scalar) so descriptor generation for
    chunk g+1 overlaps the SDMA transfer of chunk g.
  - z[b, t] = sum_k key[b,t,k] * mids[b,k] runs on VectorE as ONE fused
    scalar_tensor_tensor per [128t, 512k] tile, accumulating into z in fp32.
    No transposes of the big key tensor at all.
  - Per batch, ScalarE computes tanh (+bias) and exp (+free-axis sum). The
    softmax denominators for ALL batches are reduced/reciprocated/broadcast
    once at the end (TensorE + one tiny VectorE reciprocal), so the DVE queue
    inside the main loop carries nothing but STTs. Softmax needs no
    max-subtraction: tanh output is in (-1, 1) so exp is in (e^-1, e).
  - Output leaves in the same p-major layout via a strided 32 B-element DMA
    (64 KB total, negligible).
"""

from contextlib import ExitStack

import numpy as np

import concourse.bacc as bacc
import concourse.mybir as mybir
import concourse.tile as tile
from concourse import masks
from concourse.bass_utils import run_bass_kernel_spmd

F32 = mybir.dt.float32
MULT = mybir.AluOpType.mult
AF = mybir.ActivationFunctionType

N_CORES = 8
B, T, Q, K = 64, 2048, 512, 512
B_LOC = B // N_CORES          # 8 batches per core
N_TBLK = T // 128             # 16 [128, K] tiles per batch
CHUNK_TBLK = 8                # t-blocks per DMA chunk (8 -> 2 MB chunks)
NCHUNK = N_TBLK // CHUNK_TBLK  # DMA chunks per batch
KEY_BUFS = 4                  # key tile pool depth (4 x 2 MB = 8 MB)
ALT_QUEUES = True             # alternate key DMAs between sync/scalar HWDGE
# t-blocks per chunk routed through the TensorE path (transpose + matmul).
# Must stay 0: fp32 PE transposes measure ~475 ns sustained on HW (transpose
# mode does not count as PE-busy for the HAM clock gate, so the PE never
# leaves its low-power clock) — slower than the DVE STT they would replace.
N_PE = 0
# --- cost-model probe knobs (must be default for correctness) ---
PE_PROBE = False   # emit PE transposes but keep all z work on the STT path
STT_K = 512        # contraction length seen by the STT (probe only)
STT_FAKE = False   # STT reads mids instead of key (decouples DMA from DVE)
NO_STT = False     # skip the STT entirely (memset z once)
REPS = None        # if set, wrap the main loop in a hardware For_i (timing only)


def emit(tc, ctx):
    nc = tc.nc
    query = nc.dram_tensor("query", [B_LOC, Q], F32, kind="ExternalInput").ap()
    key = nc.dram_tensor("key", [B_LOC, T, K], F32, kind="ExternalInput").ap()
    W = nc.dram_tensor("W", [K, Q], F32, kind="ExternalInput").ap()
    bias = nc.dram_tensor("bias", [1, 1], F32, kind="ExternalInput").ap()
    out = nc.dram_tensor("out", [B_LOC, T], F32, kind="ExternalOutput").ap()

    KC = K // 128  # 4 chunks of the k axis
    QC = Q // 128  # 4 chunks of the q axis

    const = ctx.enter_context(tc.tile_pool(name="const", bufs=1))
    key_pool = ctx.enter_context(tc.tile_pool(name="keyp", bufs=KEY_BUFS))
    z_pool = ctx.enter_context(tc.tile_pool(name="zp", bufs=2))
    kT_pool = ctx.enter_context(tc.tile_pool(name="kTp", bufs=3))
    ps_setup = ctx.enter_context(tc.tile_pool(name="pss", bufs=2, space="PSUM"))
    ps_main = ctx.enter_context(tc.tile_pool(name="psm", bufs=2, space="PSUM"))
    ps_keyT = ctx.enter_context(tc.tile_pool(name="pskT", bufs=2, space="PSUM"))
    ps_z = ctx.enter_context(tc.tile_pool(name="psz", bufs=2, space="PSUM"))

    # ---- constants ----
    identity = const.tile([128, 128], F32, tag="identity")
    masks.make_identity(nc, identity[:])
    ones_col = const.tile([128, 1], F32, tag="ones_col")
    nc.vector.memset(ones_col[:], 1.0)
    ones_row = const.tile([1, 128], F32, tag="ones_row")
    nc.vector.memset(ones_row[:], 1.0)
    sel = const.tile([B_LOC, B_LOC, 128], F32, tag="sel")
    nc.gpsimd.memset(sel[:], 0.0)
    # sel[c, b, p] = 1.0 where c == b (selector columns for the mids broadcast)
    nc.gpsimd.affine_select(
        out=sel[:],
        in_=sel[:],
        compare_op=mybir.AluOpType.not_equal,
        fill=1.0,
        base=0,
        pattern=[[-1, B_LOC], [0, 128]],
        channel_multiplier=1,
    )

    # ---- small inputs (ACT HWDGE queue; key uses the sync queue first) ----
    W_sb = const.tile([128, KC, Q], F32, tag="W_sb")
    nc.scalar.dma_start(out=W_sb[:], in_=W.rearrange("(kc p) q -> p kc q", p=128))
    q_sb = const.tile([B_LOC, Q], F32, tag="q_sb")
    nc.scalar.dma_start(out=q_sb[:], in_=query)
    bias_sb = const.tile([1, 1], F32, tag="bias_sb")
    nc.scalar.dma_start(out=bias_sb[:], in_=bias)

    # Touch the activation table early (after the DMA issues, so it does not
    # delay them) — the table load overlaps the DMA pipeline fill instead of
    # the first batch's epilogue.
    warm = const.tile([1, 1], F32, tag="warm")
    nc.scalar.activation(warm[:], ones_col[:1, :], AF.Tanh)
    nc.scalar.activation(warm[:], warm[:], AF.Exp)

    # ---- W^T via TensorE transposes: WT_sb[p, qc, k] = W[k, qc*128+p] ----
    WT_sb = const.tile([128, QC, K], F32, tag="WT_sb")
    for qc in range(QC):
        wt_ps = ps_setup.tile([128, K], F32, tag="s")
        for kc in range(KC):
            nc.tensor.transpose(
                wt_ps[:, kc * 128 : (kc + 1) * 128],
                W_sb[:, kc, qc * 128 : (qc + 1) * 128],
                identity[:],
            )
        nc.scalar.copy(WT_sb[:, qc, :], wt_ps[:])

    # ---- query^T: qT_sb[p, qc, b] = query[b, qc*128+p] ----
    qT_sb = const.tile([128, QC, B_LOC], F32, tag="qT_sb")
    for qc in range(QC):
        qt_ps = ps_setup.tile([128, B_LOC], F32, tag="s")
        nc.tensor.transpose(
            qt_ps[:],
            q_sb[:, qc * 128 : (qc + 1) * 128],
            identity[:B_LOC, :B_LOC],
        )
        nc.vector.tensor_copy(qT_sb[:, qc, :], qt_ps[:])

    # ---- mids[b, k] = sum_q W[k, q] query[b, q]  (true fp32 matmul) ----
    mids_ps = ps_setup.tile([B_LOC, K], F32, tag="s")
    for qc in range(QC):
        nc.tensor.matmul(
            mids_ps[:],
            qT_sb[:, qc, :],
            WT_sb[:, qc, :],
            start=(qc == 0),
            stop=(qc == QC - 1),
        )
    mids_sb = const.tile([B_LOC, K], F32, tag="mids_sb")
    nc.scalar.copy(mids_sb[:], mids_ps[:])

    # ---- mids^T columns for the TensorE z path: midsT_sb[p, kc, b] ----
    midsT_sb = const.tile([128, KC, B_LOC], F32, tag="midsT_sb")
    for kc in range(KC):
        mt_ps = ps_setup.tile([128, B_LOC], F32, tag="s")
        nc.tensor.transpose(
            mt_ps[:],
            mids_sb[:, kc * 128 : (kc + 1) * 128],
            identity[:B_LOC, :B_LOC],
        )
        nc.vector.tensor_copy(midsT_sb[:, kc, :], mt_ps[:])

    # ---- bias broadcast to [128, 1] ----
    bb_ps = ps_setup.tile([128, 1], F32, tag="s")
    nc.tensor.matmul(bb_ps[:], ones_row[:], bias_sb[:], start=True, stop=True)
    bias_bc = const.tile([128, 1], F32, tag="bias_bc")
    nc.vector.tensor_copy(bias_bc[:], bb_ps[:])

    # ---- mids broadcast: mids_bc[p, b, k] = mids[b, k] for all p ----
    # Copies alternate ScalarE/VectorE so the setup chain halves in length.
    mids_bc = const.tile([128, B_LOC, K], F32, tag="mids_bc")
    for b in range(B_LOC):
        bc_ps = ps_setup.tile([128, K], F32, tag="s")
        nc.tensor.matmul(bc_ps[:], sel[:, b, :], mids_sb[:], start=True, stop=True)
        if b % 2 == 0:
            nc.scalar.copy(mids_bc[:, b, :], bc_ps[:])
        else:
            nc.vector.tensor_copy(mids_bc[:, b, :], bc_ps[:])

    # ---- main loop state ----
    scratch_v = const.tile([128, K], F32, tag="scratch_v")
    ex_all = const.tile([128, B_LOC, NCHUNK, CHUNK_TBLK], F32, tag="ex_all")
    exsum_all = const.tile([128, B_LOC], F32, tag="exsum_all")
    out_sb = const.tile([128, B_LOC, NCHUNK, CHUNK_TBLK], F32, tag="out_sb")
    rb_bc = const.tile([128, B_LOC], F32, tag="rb_bc")
    rec_row = const.tile([1, B_LOC], F32, tag="rec_row")

    # p-major chunk layout: t = c*(128*CHUNK_TBLK) + p*CHUNK_TBLK + n
    key_r = key.rearrange("b (c p n) k -> b c p n k", n=CHUNK_TBLK, p=128)
    out_r = out.rearrange("b (c p n) -> p b c n", n=CHUNK_TBLK, p=128)

    def main_body():
        main_loop(tc, nc, key_pool, z_pool, kT_pool, ps_main, ps_keyT, ps_z,
                  key_r, out_r, mids_bc, midsT_sb, bias_bc, scratch_v,
                  ex_all, exsum_all, out_sb,
                  rb_bc, rec_row, ones_col, ones_row, identity)

    if REPS is None:
        main_body()
    else:
        with tc.For_i(0, REPS, 1):
            main_body()


def main_loop(tc, nc, key_pool, z_pool, kT_pool, ps_main, ps_keyT, ps_z,
              key_r, out_r, mids_bc, midsT_sb, bias_bc, scratch_v,
              ex_all, exsum_all, out_sb,
              rb_bc, rec_row, ones_col, ones_row, identity):
    KC = K // 128
    g = 0
    for b in range(B_LOC):
        z_t = z_pool.tile([128, NCHUNK, CHUNK_TBLK], F32, tag="z")
        if NO_STT:
            nc.vector.memset(z_t[:], 0.5)
        for c in range(NCHUNK):
            key_t = key_pool.tile([128, CHUNK_TBLK, K], F32, tag="key")
            dma_eng = nc.sync if (not ALT_QUEUES or g % 2 == 0) else nc.scalar
            dma_eng.dma_start(out=key_t[:], in_=key_r[b, c])
            g += 1
            for n in range(CHUNK_TBLK):
                if NO_STT:
                    continue
                if n >= CHUNK_TBLK - N_PE and not STT_FAKE and STT_K == K:
                    # TensorE path: z col = keyT^T(kc-chunks) @ midsT col,
                    # accumulated over kc in PSUM.
                    keyT_ps = ps_keyT.tile([128, KC, 128], F32, tag="kT")
                    for kc in range(KC):
                        nc.tensor.transpose(
                            keyT_ps[:, kc, :],
                            key_t[:, n, kc * 128 : (kc + 1) * 128],
                            identity[:],
                        )
                    if not PE_PROBE:
                        keyT_sb = kT_pool.tile([128, KC, 128], F32, tag="kTs")
                        nc.scalar.copy(keyT_sb[:], keyT_ps[:])
                        zc_ps = ps_z.tile([128, 1], F32, tag="zc")
                        for kc in range(KC):
                            nc.tensor.matmul(
                                zc_ps[:],
                                keyT_sb[:, kc, :],
                                midsT_sb[:, kc, b : b + 1],
                                start=(kc == 0),
                                stop=(kc == KC - 1),
                            )
                        nc.scalar.copy(z_t[:, c, n : n + 1], zc_ps[:])
                        continue
                in0 = mids_bc[:, b, :STT_K] if STT_FAKE else key_t[:, n, :STT_K]
                nc.vector.scalar_tensor_tensor(
                    out=scratch_v[:, :STT_K],
                    in0=in0,
                    scalar=1.0,
                    in1=mids_bc[:, b, :STT_K],
                    op0=MULT,
                    op1=MULT,
                    accum_out=z_t[:, c, n : n + 1],
                )

        th = z_pool.tile([128, NCHUNK, CHUNK_TBLK], F32, tag="th")
        nc.scalar.activation(th[:], z_t[:], AF.Tanh, bias=bias_bc[:], scale=1.0)
        nc.scalar.activation(
            ex_all[:, b], th[:], AF.Exp, accum_out=exsum_all[:, b : b + 1]
        )
    # ---- global softmax epilogue (off the DVE critical path) ----
    # NOTE: no TensorE instruction may appear inside the batch loop above —
    # interleaving even one PE op with the STT/DMA stream serializes the loop
    # per batch on HW (~2x slowdown, measured).
    sums_ps = ps_main.tile([1, B_LOC], F32, tag="m")
    nc.tensor.matmul(sums_ps[:], ones_col[:], exsum_all[:], start=True, stop=True)
    nc.vector.reciprocal(rec_row[:], sums_ps[:])
    rb_ps = ps_main.tile([128, B_LOC], F32, tag="m")
    nc.tensor.matmul(rb_ps[:], ones_row[:], rec_row[:], start=True, stop=True)
    nc.scalar.copy(rb_bc[:], rb_ps[:])

    for b in range(B_LOC):
        nc.scalar.activation(
            out_sb[:, b], ex_all[:, b], AF.Copy, scale=rb_bc[:, b : b + 1]
        )
    nc.sync.dma_start(out=out_r, in_=out_sb[:])


_NC_CACHE = None


def build():
    global _NC_CACHE
    if _NC_CACHE is None:
        nc = bacc.Bacc(trn_type="TRN2", enable_partition_id=False)
        with tile.TileContext(nc) as tc:
            with ExitStack() as ctx:
                emit(tc, ctx)
        nc.compile()
        _NC_CACHE = nc
    return _NC_CACHE


def kernel(**inputs) -> np.ndarray:
    query = np.ascontiguousarray(np.asarray(inputs["query"], dtype=np.float32))
    key = np.ascontiguousarray(np.asarray(inputs["key"], dtype=np.float32))
    W = np.ascontiguousarray(np.asarray(inputs["W"], dtype=np.float32))
    bias = np.asarray(inputs["bias"], dtype=np.float32).reshape(1, 1)

    nc = build()
    in_maps = []
    for c in range(N_CORES):
        lo, hi = c * B_LOC, (c + 1) * B_LOC
        in_maps.append(
            {
                "query": np.ascontiguousarray(query[lo:hi]),
                "key": np.ascontiguousarray(key[lo:hi]),
                "W": W,
                "bias": bias,
            }
        )
    res = run_bass_kernel_spmd(nc, in_maps, core_ids=list(range(N_CORES)))
    return np.concatenate([res.results[c]["out"] for c in range(N_CORES)], axis=0)
